# revision 19
# baseline (speedup 1.0000x reference)
"""Trainium2 Bass kernel for a cross-modal transformer block (attention + FFN).

Contract: kernel(**inputs) takes the FULL unsharded inputs (numpy, fp32) and
returns the FULL output [4, 2048, 512] fp32.

Sharding: 8 cores = data-parallel over batch (4) x query-sequence halves (2).
Each core computes K/V projections for its batch's full 2048-token sequence
(cheap duplication) so attention needs no collectives.

Device layout: everything feature-major ([features on partitions, tokens on
free]); the host pre-transposes and pre-casts inputs so the device does zero
transposes.
"""

import functools
import sys

import numpy as np

sys.path.insert(0, "/opt/trn_rl_repo")

import ml_dtypes  # noqa: E402

import concourse.bass as bass  # noqa: E402
import concourse.tile as tile  # noqa: E402
from concourse import bacc, mybir  # noqa: E402
from concourse.bass_utils import run_bass_kernel_spmd  # noqa: E402

_orig_tables = bacc.get_activation_tables


def _patched_tables(arch):
    tabs = dict(_orig_tables(arch))
    for name in ("exp_and_others", "exp_and_friends", "natural_log"):
        if name in tabs and "natural_log_exp_and_others" in tabs:
            tabs[name] = set()
    return tabs


bacc.get_activation_tables = _patched_tables

BF16 = mybir.dt.bfloat16
F32 = mybir.dt.float32
AF = mybir.ActivationFunctionType
OP = mybir.AluOpType

# ---- custom DVE op: y = (a0 + a1*x + a2*x^2)^16 ≈ exp(SCALE_*x) for
# |SCALE_*x| <= ~3.2 (softmax logits land well inside). Runs the odd share of
# softmax exps on the vector engine so ACT and DVE split the exp wall.
import concourse.dve_ops as _dops  # noqa: E402
from concourse.dve_spec import (  # noqa: E402
    C0 as _C0,
    C1 as _C1,
    C2 as _C2,
    Spec as _Spec,
    Src0 as _Src0,
    sq as _sq,
)


def _get_exp16():
    for o in _dops.OPS:
        if o.name == "EXP16_ANT":
            return o
    op = _dops.DveOp(
        "EXP16_ANT",
        _Spec(
            body=_sq(_sq(_sq(_sq((_C2 * _Src0 + _C1) * _Src0 + _C0)))),
            reference=lambda in0, in1, s0, s1, imm2: (
                s0 + s1 * in0 + imm2 * in0 * in0
            )
            ** 16,
        ),
        subdim=False,
        uops_sha={"v3": "03babd8622902fff", "v4": "938e8369b7abe959"},
    )
    _dops.OPS.append(op)
    _dops.CUSTOM_DVE_SPECS[op.name] = op.spec
    _dops._SUB_OPCODE_FOR_NAME[op.name] = max(_dops._SUB_OPCODE_FOR_NAME.values()) + 1
    return op


EXP16 = _get_exp16()
EXP_SPLIT = True  # odd key-chunks' exp on DVE via EXP16

B, S, D = 4, 2048, 512
H, DH = 8, 64
FF = 2048
P = 128
C = D // P  # 4 feature chunks
CF = FF // P  # 16 ffn chunks
TQ = S // 2  # 1024 query tokens per core
TK = S  # full key sequence per core
KC = TK // P  # 16 key chunks
NT = 512  # token tile (matmul free dim)
NQ = TQ // NT  # 2 query-token tiles
SCALE = 1.0 / np.sqrt(DH)  # 0.125
LN_EPS = 1e-5
NCORES = 8

# exp(SCALE*x) ~ (EA0 + EA1*x + EA2*x^2)^16 coefficients (minimax on
# scaled-logit range +-3.2; max rel err 5.4e-3)
EA0 = 1.00004971
EA1 = 1.00498309 * (SCALE / 16.0)
EA2 = 0.49875139 * (SCALE / 16.0) ** 2
DVE_KC = frozenset({2, 5, 8, 11, 14})  # key-chunks whose exp runs on DVE


def _emit(nc, t, es, tc):
    """Emit the per-core program. t: dict name -> DRAM AP."""
    # ---------------- pools ----------------
    pw = es.enter_context(tc.tile_pool(name="projw", bufs=1))
    wp = es.enter_context(tc.tile_pool(name="w", bufs=1))
    ap_ = es.enter_context(tc.tile_pool(name="acts", bufs=1))
    ptq = es.enter_context(tc.tile_pool(name="ptq", bufs=2))
    psS = es.enter_context(tc.tile_pool(name="psS", bufs=2, space="PSUM"))
    psC = es.enter_context(tc.tile_pool(name="psC", bufs=2, space="PSUM"))
    psX = es.enter_context(tc.tile_pool(name="psX", bufs=2, space="PSUM"))
    stream = es.enter_context(tc.tile_pool(name="stream", bufs=6))
    vpool = es.enter_context(tc.tile_pool(name="vpool", bufs=5))
    stage = es.enter_context(tc.tile_pool(name="stage", bufs=2))
    stage1 = es.enter_context(tc.tile_pool(name="stage1", bufs=1))
    chunk = es.enter_context(tc.tile_pool(name="chunk", bufs=2))
    small = es.enter_context(tc.tile_pool(name="small", bufs=6))
    epool = es.enter_context(tc.tile_pool(name="e", bufs=6))
    hpool = es.enter_context(tc.tile_pool(name="h", bufs=1))

    def ld_w(pool, name, kchunks, n):
        w = pool.tile([P, kchunks, n], BF16, name=name + "_sb")
        src_ = t[name].rearrange("p (c o) -> p c o", c=kchunks)
        for ki in range(kchunks):
            nc.sync.dma_start(w[:, ki, :], src_[:, ki, :])
        return w

    # all small per-feature vectors arrive pre-shuffled in one [P, 48] pack
    ball = wp.tile([P, 48], F32, name="ball")
    nc.sync.dma_start(ball, t["ball"])
    bq, bk, bo, b2 = (ball[:, 4 * i : 4 * (i + 1)] for i in range(4))
    g1, be1, g2, be2 = (ball[:, 16 + 4 * i : 20 + 4 * i] for i in range(4))
    b1 = ball[:, 32:48]

    wk = ld_w(pw, "wk", C, D)
    krs = {}
    for half in range(2):
        kr = []
        for ki in range(C):
            r = stream.tile([P, 1024], BF16, tag="xr", name=f"kr_{half}_{ki}")
            xk_ = t["xkb"].rearrange("p (c q) -> p c q", c=C)
            hs = slice(half * 1024, (half + 1) * 1024)
            nc.sync.dma_start(r[0:64], xk_[0:64, ki, hs])
            nc.sync.dma_start(r[64:P], xk_[64:P, ki, hs])
            kr.append(r)
        krs[half] = kr
    wv = ld_w(pw, "wv", C, D)
    wq = ld_w(pw, "wq", C, D)

    wo = ld_w(wp, "wo", C, D)
    w1d = t["w1"].rearrange("p (c o) -> p c o", c=C)
    w2d = t["w2"].rearrange("p (c o) -> p c o", c=CF)

    ones = wp.tile([P, 1], F32)
    nc.vector.memset(ones, 1.0)
    epst = wp.tile([1, 1], F32)
    nc.vector.memset(epst, LN_EPS)

    # persistent activations (full key sequence); tags shared with the FFN
    # weights, which reuse these slots once attention is done
    kts = [ap_.tile([P, TK], BF16, tag=f"big1_{i}", name=f"kt_{i}") for i in range(C)]
    # V token-major, with a 65th all-ones column per head so the ctx matmul
    # emits sumexp on PSUM row 64 for free
    va = ap_.tile([P, KC, H, DH + 1], BF16, tag="big2", name="va")
    nc.vector.memset(va[:, :, :, DH], 1.0)

    xq32d = t["xq32"].rearrange("p (c q) -> p c q", c=C)
    xqbd = t["xqb"].rearrange("p (c q) -> p c q", c=C)
    xkb = t["xkb"].rearrange("p (c q) -> p c q", c=C)
    xvb = t["xvb"].rearrange("p (c q) -> p c q", c=C)
    out_d = t["out"].rearrange("(c p) q -> p c q", p=P)

    # ---------------- P1: K/V projections + Q proj (tile 0) ----------------
    for half in range(2):
        kr = krs[half]
        for tk in range(2):
            ts_ = slice(half * 1024 + tk * NT, half * 1024 + (tk + 1) * NT)
            tsl = slice(tk * NT, (tk + 1) * NT)
            for co in range(C):
                ps = psC.tile([P, NT], F32, tag="pc", name=f"kps_{half}_{tk}_{co}")
                for ki in range(C):
                    nc.tensor.matmul(
                        ps,
                        wk[:, ki, co * P : (co + 1) * P],
                        kr[ki][:, tsl],
                        start=(ki == 0),
                        stop=(ki == C - 1),
                    )
                nc.vector.tensor_scalar(
                    out=kts[co][:, ts_], in0=ps, scalar1=bk[:, co : co + 1],
                    scalar2=None, op0=OP.add,
                )

    vrs = {}

    def load_vr(half):
        hs = slice(half * 1024, (half + 1) * 1024)
        vr = []
        for ki in range(C):
            r = vpool.tile([P, 1024], BF16, tag="vr", name=f"vr_{half}_{ki}")
            nc.sync.dma_start(r[0:64], xvb[0:64, ki, hs])
            nc.sync.dma_start(r[64:P], xvb[64:P, ki, hs])
            vr.append(r)
        vrs[half] = vr

    def vproj_chunk(tm):
        half, tm8 = divmod(tm, 8)
        msl = slice(tm8 * P, (tm8 + 1) * P)
        vr = vrs[half]
        ps = psC.tile([P, NT], F32, tag="pc", name=f"vps_{tm}")
        for ki in range(C):
            nc.tensor.matmul(
                ps, vr[ki][:, msl], wv[:, ki, :],
                start=(ki == 0), stop=(ki == C - 1),
            )
        # bv is folded into bo on the host, so this is a pure copy
        nc.vector.tensor_copy(
            out=va[:, tm, :, 0:DH],
            in_=ps.rearrange("p (h d) -> p h d", h=H),
        )

    qr = []
    for ki in range(C):
        r = stream.tile([P, TQ], BF16, tag="xr", name=f"qr_{ki}")
        nc.sync.dma_start(r[0:64], xqbd[0:64, ki, :])
        nc.sync.dma_start(r[64:P], xqbd[64:P, ki, :])
        qr.append(r)

    def qproj_co(tq, co):
        tsl = slice(tq * NT, (tq + 1) * NT)
        qt = qts[tq]
        ps = psC.tile([P, NT], F32, tag="pc", name=f"qps_{tq}_{co}")
        for ki in range(C):
            nc.tensor.matmul(
                ps,
                wq[:, ki, co * P : (co + 1) * P],
                qr[ki][:, tsl],
                start=(ki == 0),
                stop=(ki == C - 1),
            )
        nc.vector.tensor_scalar(
            out=qt[:, co, :], in0=ps, scalar1=bq[:, co : co + 1],
            scalar2=None, op0=OP.add,
        )

    def qproj(tq):
        qts[tq] = ptq.tile([P, C, NT], BF16, tag="qt", name=f"qt_{tq}")
        for co in range(C):
            qproj_co(tq, co)
        return qts[tq]

    load_vr(0)
    qts = [None, None]
    qproj(0)

    # ---------------- attention helpers ----------------
    def attn_hp(tq, qt, ctx, hp, filler=None, dve_kcs=frozenset()):
        """One head pair's attention, software-pipelined over key chunks.
        filler(kc) lets independent work ride along inside the loop.
        ctx+sumexp are fused: the 65-col V (ones in col 64) makes each head's
        ctx matmul deposit sumexp on PSUM row 64. Key chunks in dve_kcs get
        their exp on the vector engine (EXP16) instead of ACT."""
        pcs = [
            psX.tile([DH + 1, NT], F32, tag="pcx", name=f"pcc_{tq}_{hp}_{j}")
            for j in range(2)
        ]
        e2s = [None] * KC
        for kc in range(KC + 1):
            if kc < KC:
                ksl = slice(kc * P, (kc + 1) * P)
                ps2 = psS.tile([P, 2, NT], F32, tag="ps2", name=f"sps_{tq}_{hp}_{kc}")
                e2 = epool.tile([P, 2, NT], BF16, tag="e", name=f"e_{tq}_{hp}_{kc}")
                for j in range(2):  # head 2*hp + j at partition offset 64*j
                    rows = slice(j * DH, (j + 1) * DH)
                    nc.tensor.matmul(
                        ps2[:, j, :], kts[hp][rows, ksl], qt[rows, hp, :],
                        start=True, stop=True,
                    )
                if EXP_SPLIT and kc in dve_kcs:
                    nc.vector._custom_dve(
                        EXP16,
                        out=e2.rearrange("p a n -> p (a n)"),
                        in0=ps2.rearrange("p a n -> p (a n)"),
                        s0=EA0, s1=EA1, imm2=EA2,
                    )
                else:
                    nc.scalar.activation(e2, ps2, AF.Exp, scale=SCALE)
                e2s[kc] = e2
            if filler is not None and kc < KC:
                filler(kc)
            if kc >= 1:
                st, sp = (kc - 1 == 0), (kc - 1 == KC - 1)
                for j in range(2):
                    nc.tensor.matmul(
                        pcs[j],
                        va[:, kc - 1, 2 * hp + j, :],
                        e2s[kc - 1][:, j, :],
                        start=st, stop=sp,
                        skip_group_check=True,
                    )
        for j in range(2):
            # custom-DVE recip can't read PSUM at partition offset 64 on HW;
            # bounce the sumexp row through SBUF first
            se = small.tile([1, NT], F32, tag="sm", name=f"se_{tq}_{hp}_{j}")
            nc.vector.tensor_copy(out=se, in_=pcs[j][DH : DH + 1, :])
            rc = small.tile([1, NT], F32, tag="sm", name=f"rc_{tq}_{hp}_{j}")
            nc.vector.reciprocal_approx_fast(out=rc, in_=se)
            db = chunk.tile([DH, NT], F32, tag="db", name=f"db_{tq}_{hp}_{j}")
            nc.gpsimd.partition_broadcast(db, rc)
            nc.vector.tensor_tensor(
                out=ctx[j * DH : (j + 1) * DH, hp, :],
                in0=pcs[j][0:DH, :],
                in1=db,
                op=OP.mult,
            )

    def oproj_co(tq, ctx, resid, co):
        ts_ = slice(tq * NT, (tq + 1) * NT)
        xqc = chunk.tile([P, NT], F32, tag="xqc", name=f"xqc_{tq}_{co}")
        nc.sync.dma_start(xqc, xq32d[:, co, ts_])
        ps = psC.tile([P, NT], F32, tag="pc", name=f"ops_{tq}_{co}")
        for ki in range(C):
            nc.tensor.matmul(
                ps,
                wo[:, ki, co * P : (co + 1) * P],
                ctx[:, ki, :],
                start=(ki == 0),
                stop=(ki == C - 1),
            )
        nc.vector.scalar_tensor_tensor(
            out=resid[:, co, :],
            in0=ps,
            scalar=bo[:, co : co + 1],
            in1=xqc,
            op0=OP.add,
            op1=OP.add,
        )

    def layernorm(resid, g, be, out_write, tag, out_write_co=None):
        """Normalizes resid IN PLACE (except the final +be which out_write
        directs). resid: [P, C, NT] f32 tile."""
        lnp = psC.tile([P, NT], F32, tag="pc", name=f"lnp_{tag}")
        for co in range(C):
            nc.tensor.matmul(lnp[0:1, :], ones, resid[:, co, :], start=(co == 0),
                             stop=(co == C - 1), skip_group_check=True)
        s4 = stage1.tile([P, C, NT], F32, tag="sq", name=f"sq_{tag}")
        nc.vector.tensor_mul(s4, resid, resid)
        for co in range(C):
            nc.tensor.matmul(lnp[64:65, :], ones, s4[:, co, :], start=(co == 0),
                             stop=(co == C - 1), tile_position=(0, 64),
                             skip_group_check=True)
        mean = small.tile([1, NT], F32, tag="sm", name=f"mean_{tag}")
        nc.vector.tensor_scalar_mul(mean, lnp[0:1, :], 1.0 / D)
        msq = small.tile([1, NT], F32, tag="sm", name=f"msq_{tag}")
        nc.vector.tensor_scalar_mul(msq, lnp[64:65, :], 1.0 / D)
        m2 = small.tile([1, NT], F32, tag="sm", name=f"m2_{tag}")
        nc.vector.tensor_mul(m2, mean, mean)
        var = small.tile([1, NT], F32, tag="sm", name=f"var_{tag}")
        nc.vector.tensor_tensor(out=var, in0=msq, in1=m2, op=OP.subtract)
        # rstd = exp(-0.5 * ln(var + eps)) -- stays in the Exp/Ln ACT table set
        lnv = small.tile([1, NT], F32, tag="sm", name=f"lnv_{tag}")
        nc.scalar.activation(lnv, var, AF.Ln, bias=epst)
        rstd = small.tile([1, NT], F32, tag="sm", name=f"rstd_{tag}")
        nc.scalar.activation(rstd, lnv, AF.Exp, scale=-0.5)
        meanb = chunk.tile([P, NT], F32, tag="bc", name=f"meanb_{tag}")
        nc.gpsimd.partition_broadcast(meanb, mean)
        rstdb = chunk.tile([P, NT], F32, tag="bc", name=f"rstdb_{tag}")
        nc.gpsimd.partition_broadcast(rstdb, rstd)
        if out_write_co is not None:
            for co in range(C):
                nc.vector.tensor_tensor(
                    out=resid[:, co, :], in0=resid[:, co, :], in1=meanb,
                    op=OP.subtract,
                )
                nc.vector.scalar_tensor_tensor(
                    out=resid[:, co, :], in0=resid[:, co, :],
                    scalar=g[:, co : co + 1], in1=rstdb, op0=OP.mult, op1=OP.mult,
                )
                out_write_co(co, resid[:, co, :], be[:, co : co + 1])
            return
        nc.vector.tensor_tensor(
            out=resid, in0=resid,
            in1=meanb[:, None, :].to_broadcast((P, C, NT)), op=OP.subtract,
        )
        nc.vector.tensor_tensor(
            out=resid, in0=resid,
            in1=rstdb[:, None, :].to_broadcast((P, C, NT)), op=OP.mult,
        )
        nc.vector.tensor_tensor(
            out=resid, in0=resid,
            in1=g[:, :, None].to_broadcast((P, C, NT)), op=OP.mult,
        )
        out_write(resid, be)

    # ---------------- P2: attention(tq0) with V-proj / Qproj(tq1) riding --
    ctx0 = ptq.tile([P, C, NT], BF16, tag="ctx", name="ctx_0")

    def fill_hp0(kc):
        # V chunk kc feeds this very head pair's ctx matmuls a beat later
        if kc == 2:
            load_vr(1)
        vproj_chunk(kc)

    def fill_hp2(kc):
        if kc % 4 == 0:
            qproj_co(1, kc // 4)

    # per-hp DVE exp assignment: lighter where DVE-heavy riders coexist
    DK0 = [
        frozenset(),  # hp0: vproj evacuations ride on DVE
        frozenset({1, 4, 7, 10, 13}),
        frozenset({2, 5, 8, 11}),  # qproj(1) rides
        frozenset({1, 4, 7, 10, 13}),
    ]
    for hp in range(H // 2):
        attn_hp(0, qts[0], ctx0, hp,
                filler=(fill_hp0 if hp == 0 else fill_hp2 if hp == 2 else None),
                dve_kcs=DK0[hp])
        if hp == 1:
            qts[1] = ptq.tile([P, C, NT], BF16, tag="qt", name="qt_1")

    # ---------------- P3: attention(tq1) + Oproj(0)/LN1(0) interleaved ----
    ctx1 = ptq.tile([P, C, NT], BF16, tag="ctx", name="ctx_1")
    resid0 = stage.tile([P, C, NT], F32, tag="resid", name="resid_0")
    w1s = []
    ln1f0 = stage.tile([P, C, NT], F32, tag="ln1f", name="ln1f_0")
    ln1b0 = ptq.tile([P, C, NT], BF16, tag="ln1b", name="ln1b_0")

    def write_ln1_0(tt, be):
        nc.vector.tensor_tensor(
            out=ln1f0, in0=tt,
            in1=be[:, :, None].to_broadcast((P, C, NT)), op=OP.add,
        )
        nc.vector.tensor_copy(out=ln1b0, in_=ln1f0)

    DK1 = [
        frozenset({1, 4, 7, 10, 13}),
        frozenset({1, 4, 7, 10, 13}),
        frozenset({2, 6, 10}),  # LN1(0) rides on DVE
        frozenset({1, 4, 7, 10, 13}),
    ]
    for hp in range(H // 2):
        attn_hp(1, qts[1], ctx1, hp, dve_kcs=DK1[hp])
        # kts[hp] is now dead; its slot hosts the matching quarter of W1
        w1q = ap_.tile([P, FF], BF16, tag=f"big1_{hp}", name=f"w1s_{hp}")
        nc.sync.dma_start(w1q[:, 0 : FF // 2], w1d[:, hp, 0 : FF // 2])
        nc.sync.dma_start(w1q[:, FF // 2 :], w1d[:, hp, FF // 2 :])
        w1s.append(w1q)
        if hp < 2:
            # both Oproj(0) chunks per early head pair
            oproj_co(0, ctx0, resid0, 2 * hp)
            oproj_co(0, ctx0, resid0, 2 * hp + 1)
        elif hp == 2:
            # LN1(0): its serial chain hides under the last head pair
            layernorm(resid0, g1, be1, write_ln1_0, "l1_0")

    # ---------------- P4: W2 (reusing the va slot) + tails ----------------
    w2s = ap_.tile([P, CF, D], BF16, tag="big2", name="w2s")
    for kq in range(4):
        nc.sync.dma_start(w2s[:, 4 * kq : 4 * kq + 4, :], w2d[:, 4 * kq : 4 * kq + 4, :])

    # Oproj(1) first; each LN chain then overlaps the next FFN's matmuls
    resid1 = stage.tile([P, C, NT], F32, tag="resid", name="resid_1")
    for co in range(C):
        oproj_co(1, ctx1, resid1, co)

    ln1f1 = stage.tile([P, C, NT], F32, tag="ln1f", name="ln1f_1")
    ln1b1 = ptq.tile([P, C, NT], BF16, tag="ln1b", name="ln1b_1")

    def write_ln1_1(tt, be):
        nc.vector.tensor_tensor(
            out=ln1f1, in0=tt,
            in1=be[:, :, None].to_broadcast((P, C, NT)), op=OP.add,
        )
        nc.vector.tensor_copy(out=ln1b1, in_=ln1f1)

    hbs = [None, None]

    def ffn1(tq, ln1b):
        hb = hpool.tile([P, CF, NT], BF16, tag="h", name=f"h_{tq}")
        for fo in range(CF):
            ps = psC.tile([P, NT], F32, tag="pc", name=f"fps_{tq}_{fo}")
            for ki in range(C):
                nc.tensor.matmul(
                    ps,
                    w1s[ki][:, fo * P : (fo + 1) * P],
                    ln1b[:, ki, :],
                    start=(ki == 0),
                    stop=(ki == C - 1),
                )
            nc.scalar.activation(hb[:, fo, :], ps, AF.Gelu, bias=b1[:, fo : fo + 1])
        hbs[tq] = hb

    def ffn2(tq, ln1f):
        hb = hbs[tq]
        resid2 = stage1.tile([P, C, NT], F32, tag="resid2", name=f"resid2_{tq}")
        for co in range(C):
            ps = psC.tile([P, NT], F32, tag="pc", name=f"gps_{tq}_{co}")
            for ki in range(CF):
                nc.tensor.matmul(
                    ps,
                    w2s[:, ki, co * P : (co + 1) * P],
                    hb[:, ki, :],
                    start=(ki == 0),
                    stop=(ki == CF - 1),
                )
            nc.vector.scalar_tensor_tensor(
                out=resid2[:, co, :],
                in0=ps,
                scalar=b2[:, co : co + 1],
                in1=ln1f[:, co, :],
                op0=OP.add,
                op1=OP.add,
            )
        return resid2

    def ln2(tq, resid2):
        ts_ = slice(tq * NT, (tq + 1) * NT)

        def write_out_co(co, v, bec, ts_=ts_):
            nc.vector.tensor_scalar(
                out=v, in0=v, scalar1=bec, scalar2=None, op0=OP.add
            )
            nc.sync.dma_start(out_d[:, co, ts_], v)

        layernorm(resid2, g2, be2, None, f"l2_{tq}", out_write_co=write_out_co)

    ffn1(0, ln1b0)                     # fills the Oproj(1) tail
    layernorm(resid1, g1, be1, write_ln1_1, "l1_1")  # chain hides in FFN2(0)
    r2_0 = ffn2(0, ln1f0)
    ln2(0, r2_0)                       # chain hides in FFN1(1)
    ffn1(1, ln1b1)
    r2_1 = ffn2(1, ln1f1)
    ln2(1, r2_1)


@functools.lru_cache(maxsize=1)
def build():
    from contextlib import ExitStack

    nc = bacc.Bacc("TRN2", target_bir_lowering=False, debug=False, num_devices=NCORES)
    t = {}

    def din(name, shape, dt):
        t[name] = nc.dram_tensor(name, list(shape), dt, kind="ExternalInput").ap()

    din("xq32", (P, C * TQ), F32)
    din("xqb", (P, C * TQ), BF16)
    din("xkb", (P, C * TK), BF16)
    din("xvb", (P, C * TK), BF16)
    for w in ("wq", "wk", "wv", "wo"):
        din(w, (P, C * D), BF16)
    din("w1", (P, C * FF), BF16)
    din("w2", (P, CF * D), BF16)
    din("ball", (P, 48), F32)
    t["out"] = nc.dram_tensor("out", [D, TQ], F32, kind="ExternalOutput").ap()

    with tile.TileContext(nc) as tc:
        with ExitStack() as es:
            _emit(nc, t, es, tc)
    nc.compile()
    return nc


def make_in_maps(query, key, value, Wq, bq, Wk, bk, Wv, bv, Wo, bo,
                 g1, be1, g2, be2, W1, b1, W2, b2):
    bf = ml_dtypes.bfloat16

    def pmaj(w, dt=bf):
        # [K, N] -> partition-major [128, (K//128) * N], contiguous rows
        w = np.asarray(w)
        k, n = w.shape
        return np.ascontiguousarray(
            w.reshape(k // P, P, n).transpose(1, 0, 2).reshape(P, -1).astype(dt)
        )

    # bv is softmax-invariant through attention: softmax(s)@(V+bv) =
    # softmax(s)@V + bv, so fold it through Wo into bo.
    bo_eff = np.asarray(bv, np.float32) @ np.asarray(Wo, np.float32) + np.asarray(
        bo, np.float32
    )
    cols = [np.asarray(v, np.float32).reshape(-1, P).T for v in (bq, bk, bo_eff, b2, g1, be1, g2, be2, b1)]
    ball = np.ascontiguousarray(np.concatenate(cols, axis=1))  # [128, 48]
    shared = {
        "ball": ball,
        "wq": pmaj(Wq), "wk": pmaj(Wk), "wv": pmaj(Wv), "wo": pmaj(Wo),
        "w1": pmaj(W1), "w2": pmaj(W2),
    }
    in_maps = []
    for core in range(NCORES):
        b, half = divmod(core, 2)
        qsl = slice(half * TQ, (half + 1) * TQ)
        xq_t = np.asarray(query[b, qsl], np.float32).T  # [D, TQ]
        in_maps.append({
            "xq32": pmaj(xq_t, np.float32), "xqb": pmaj(xq_t),
            "xkb": pmaj(np.asarray(key[b], np.float32).T),
            "xvb": pmaj(np.asarray(value[b], np.float32).T), **shared,
        })
    return in_maps


def kernel(**inputs):
    nc = build()
    in_maps = make_in_maps(**inputs)
    res = run_bass_kernel_spmd(nc, in_maps, list(range(NCORES)))
    out = np.empty((B, S, D), np.float32)
    for core in range(NCORES):
        b, half = divmod(core, 2)
        out[b, half * TQ : (half + 1) * TQ] = res.results[core]["out"].T
    return out


if __name__ == "__main__":
    import reference

    inputs = {k: np.asarray(v) for k, v in reference.setup_inputs().items()}
    got = kernel(**inputs)
    exp = np.asarray(reference.reference(**inputs))
    err = np.abs(got - exp).max() / np.abs(exp).max()
    print("rel err:", err)



# revision 29
# speedup vs baseline: 1.0670x; 1.0670x over previous
"""Trainium2 Bass kernel for a cross-modal transformer block (attention + FFN).

Contract: kernel(**inputs) takes the FULL unsharded inputs (numpy, fp32) and
returns the FULL output [4, 2048, 512] fp32.

Sharding: 8 cores = data-parallel over batch (4) x query-sequence halves (2).
Each core computes K/V projections for its batch's full 2048-token sequence
(cheap duplication) so attention needs no collectives.

Device layout: everything feature-major ([features on partitions, tokens on
free]); the host pre-transposes and pre-casts inputs so the device does zero
transposes.
"""

import functools
import sys

import numpy as np

sys.path.insert(0, "/opt/trn_rl_repo")

import ml_dtypes  # noqa: E402

import concourse.bass as bass  # noqa: E402
import concourse.tile as tile  # noqa: E402
from concourse import bacc, mybir  # noqa: E402
from concourse.bass_utils import run_bass_kernel_spmd  # noqa: E402

_orig_tables = bacc.get_activation_tables


def _patched_tables(arch):
    tabs = dict(_orig_tables(arch))
    for name in ("exp_and_others", "exp_and_friends", "natural_log"):
        if name in tabs and "natural_log_exp_and_others" in tabs:
            tabs[name] = set()
    return tabs


bacc.get_activation_tables = _patched_tables

BF16 = mybir.dt.bfloat16
F32 = mybir.dt.float32
AF = mybir.ActivationFunctionType
OP = mybir.AluOpType

# ---- custom DVE op: y = (a0 + a1*x + a2*x^2)^16 ≈ exp(SCALE_*x) for
# |SCALE_*x| <= ~3.2 (softmax logits land well inside). Runs the odd share of
# softmax exps on the vector engine so ACT and DVE split the exp wall.
import concourse.dve_ops as _dops  # noqa: E402
from concourse.dve_spec import (  # noqa: E402
    C0 as _C0,
    C1 as _C1,
    C2 as _C2,
    Spec as _Spec,
    Src0 as _Src0,
    sq as _sq,
)


def _get_exp16():
    for o in _dops.OPS:
        if o.name == "EXP16_ANT":
            return o
    op = _dops.DveOp(
        "EXP16_ANT",
        _Spec(
            body=_sq(_sq(_sq(_sq((_C2 * _Src0 + _C1) * _Src0 + _C0)))),
            reference=lambda in0, in1, s0, s1, imm2: (
                s0 + s1 * in0 + imm2 * in0 * in0
            )
            ** 16,
        ),
        subdim=False,
        uops_sha={"v3": "03babd8622902fff", "v4": "938e8369b7abe959"},
    )
    _dops.OPS.append(op)
    _dops.CUSTOM_DVE_SPECS[op.name] = op.spec
    _dops._SUB_OPCODE_FOR_NAME[op.name] = max(_dops._SUB_OPCODE_FOR_NAME.values()) + 1
    return op


EXP16 = _get_exp16()
# NOTE: splitting exp onto DVE saturates all four engines at once and trips
# the chip power throttle (~17% clock cut on everything) — measured net loss.
EXP_SPLIT = False

B, S, D = 4, 2048, 512
H, DH = 8, 64
FF = 2048
P = 128
C = D // P  # 4 feature chunks
CF = FF // P  # 16 ffn chunks
TQ = S // 2  # 1024 query tokens per core
TK = S  # full key sequence per core
KC = TK // P  # 16 key chunks
NT = 512  # token tile (matmul free dim)
NQ = TQ // NT  # 2 query-token tiles
SCALE = 1.0 / np.sqrt(DH)  # 0.125
LN_EPS = 1e-5
NCORES = 8

# exp(SCALE*x) ~ (EA0 + EA1*x + EA2*x^2)^16 coefficients (minimax on
# scaled-logit range +-3.2; max rel err 5.4e-3)
EA0 = 1.00004971
EA1 = 1.00498309 * (SCALE / 16.0)
EA2 = 0.49875139 * (SCALE / 16.0) ** 2
DVE_KC = frozenset({2, 5, 8, 11, 14})  # key-chunks whose exp runs on DVE


def _emit(nc, t, es, tc):
    """Emit the per-core program. t: dict name -> DRAM AP."""
    # ---------------- pools ----------------
    pw = es.enter_context(tc.tile_pool(name="projw", bufs=1))
    wp = es.enter_context(tc.tile_pool(name="w", bufs=1))
    ap_ = es.enter_context(tc.tile_pool(name="acts", bufs=1))
    ptq = es.enter_context(tc.tile_pool(name="ptq", bufs=2))
    psS = es.enter_context(tc.tile_pool(name="psS", bufs=2, space="PSUM"))
    psC = es.enter_context(tc.tile_pool(name="psC", bufs=2, space="PSUM"))
    psX = es.enter_context(tc.tile_pool(name="psX", bufs=2, space="PSUM"))
    krpool = es.enter_context(tc.tile_pool(name="krpool", bufs=8))
    qrpool = es.enter_context(tc.tile_pool(name="qrpool", bufs=4))
    vpool = es.enter_context(tc.tile_pool(name="vpool", bufs=5))
    stage = es.enter_context(tc.tile_pool(name="stage", bufs=2))
    stage1 = es.enter_context(tc.tile_pool(name="stage1", bufs=1))
    chunk = es.enter_context(tc.tile_pool(name="chunk", bufs=2))
    small = es.enter_context(tc.tile_pool(name="small", bufs=6))
    epool = es.enter_context(tc.tile_pool(name="e", bufs=6))
    hpool = es.enter_context(tc.tile_pool(name="h", bufs=1))

    def ld_w(pool, name, kchunks, n):
        w = pool.tile([P, kchunks, n], BF16, name=name + "_sb")
        src_ = t[name].rearrange("p (c o) -> p c o", c=kchunks)
        for ki in range(kchunks):
            nc.sync.dma_start(w[:, ki, :], src_[:, ki, :])
        return w

    # all small per-feature vectors arrive pre-shuffled in one [P, 48] pack
    ball = wp.tile([P, 48], F32, name="ball")
    nc.sync.dma_start(ball, t["ball"])
    bq, bk, bo, b2 = (ball[:, 4 * i : 4 * (i + 1)] for i in range(4))
    g1, be1, g2, be2 = (ball[:, 16 + 4 * i : 20 + 4 * i] for i in range(4))
    b1 = ball[:, 32:48]

    wk = ld_w(pw, "wk", C, D)
    krs = {}
    for half in range(2):
        kr = []
        for ki in range(C):
            r = krpool.tile([P, 1024], BF16, tag="kr", name=f"kr_{half}_{ki}")
            xk_ = t["xkb"].rearrange("p (c q) -> p c q", c=C)
            hs = slice(half * 1024, (half + 1) * 1024)
            nc.sync.dma_start(r[0:64], xk_[0:64, ki, hs])
            nc.sync.dma_start(r[64:P], xk_[64:P, ki, hs])
            kr.append(r)
        krs[half] = kr
    wv = ld_w(pw, "wv", C, D)
    wq = ld_w(pw, "wq", C, D)

    wo = ld_w(wp, "wo", C, D)
    w1d = t["w1"].rearrange("p (c o) -> p c o", c=C)
    w2d = t["w2"].rearrange("p (c o) -> p c o", c=CF)

    ones = wp.tile([P, 1], F32)
    nc.vector.memset(ones, 1.0)
    epst = wp.tile([1, 1], F32)
    nc.vector.memset(epst, LN_EPS)

    # persistent activations (full key sequence); tags shared with the FFN
    # weights, which reuse these slots once attention is done
    kts = [ap_.tile([P, TK], BF16, tag=f"big1_{i}", name=f"kt_{i}") for i in range(C)]
    # V token-major, with a 65th all-ones column per head so the ctx matmul
    # emits sumexp on PSUM row 64 for free
    va = ap_.tile([P, KC, H, DH + 1], BF16, tag="big2", name="va")
    nc.vector.memset(va[:, :, :, DH], 1.0)

    xq32d = t["xq32"].rearrange("p (c q) -> p c q", c=C)
    xqbd = t["xqb"].rearrange("p (c q) -> p c q", c=C)
    xkb = t["xkb"].rearrange("p (c q) -> p c q", c=C)
    xvb = t["xvb"].rearrange("p (c q) -> p c q", c=C)
    out_d = t["out"].rearrange("(c p) q -> p c q", p=P)

    # ---------------- K projection, one feature chunk (co) at a time -------
    # co=0 runs up front; co=1..3 ride inside attn(0) as quarter-granular
    # fillers just ahead of the head pair that consumes them.
    def kproj_quarter(co, q):
        half, tk = divmod(q, 2)
        kr = krs[half]
        ts_ = slice(half * 1024 + tk * NT, half * 1024 + (tk + 1) * NT)
        tsl = slice(tk * NT, (tk + 1) * NT)
        ps = psC.tile([P, NT], F32, tag="pc", name=f"kps_{co}_{q}")
        for ki in range(C):
            nc.tensor.matmul(
                ps,
                wk[:, ki, co * P : (co + 1) * P],
                kr[ki][:, tsl],
                start=(ki == 0),
                stop=(ki == C - 1),
            )
        nc.vector.tensor_scalar(
            out=kts[co][:, ts_], in0=ps, scalar1=bk[:, co : co + 1],
            scalar2=None, op0=OP.add,
        )

    vrs = {}

    def load_vr(half):
        hs = slice(half * 1024, (half + 1) * 1024)
        vr = []
        for ki in range(C):
            r = vpool.tile([P, 1024], BF16, tag="vr", name=f"vr_{half}_{ki}")
            nc.sync.dma_start(r[0:64], xvb[0:64, ki, hs])
            nc.sync.dma_start(r[64:P], xvb[64:P, ki, hs])
            vr.append(r)
        vrs[half] = vr

    def vproj_chunk(tm):
        half, tm8 = divmod(tm, 8)
        msl = slice(tm8 * P, (tm8 + 1) * P)
        vr = vrs[half]
        ps = psC.tile([P, NT], F32, tag="pc", name=f"vps_{tm}")
        for ki in range(C):
            nc.tensor.matmul(
                ps, vr[ki][:, msl], wv[:, ki, :],
                start=(ki == 0), stop=(ki == C - 1),
            )
        # bv is folded into bo on the host, so this is a pure copy
        nc.vector.tensor_copy(
            out=va[:, tm, :, 0:DH],
            in_=ps.rearrange("p (h d) -> p h d", h=H),
        )

    qr = []
    for ki in range(C):
        r = qrpool.tile([P, TQ], BF16, tag="qr", name=f"qr_{ki}")
        nc.sync.dma_start(r[0:64], xqbd[0:64, ki, :])
        nc.sync.dma_start(r[64:P], xqbd[64:P, ki, :])
        qr.append(r)

    def qproj_co(tq, co):
        tsl = slice(tq * NT, (tq + 1) * NT)
        qt = qts[tq]
        ps = psC.tile([P, NT], F32, tag="pc", name=f"qps_{tq}_{co}")
        for ki in range(C):
            nc.tensor.matmul(
                ps,
                wq[:, ki, co * P : (co + 1) * P],
                qr[ki][:, tsl],
                start=(ki == 0),
                stop=(ki == C - 1),
            )
        nc.vector.tensor_scalar(
            out=qt[:, co, :], in0=ps, scalar1=bq[:, co : co + 1],
            scalar2=None, op0=OP.add,
        )

    load_vr(0)
    qts = [None, None]
    qts[0] = ptq.tile([P, C, NT], BF16, tag="qt", name="qt_0")
    # lead-in: only K feature-chunk 0 + Q(0) chunk 0 — the minimum attn(0)
    # hp0 needs; the other chunks ride inside the attention loops
    for q in range(4):
        kproj_quarter(0, q)
    qproj_co(0, 0)

    # ---------------- attention helpers ----------------
    def attn_hp(tq, qt, ctx, hp, filler=None, dve_kcs=frozenset()):
        """One head pair's attention, software-pipelined over key chunks.
        filler(kc) lets independent work ride along inside the loop.
        ctx+sumexp are fused: the 65-col V (ones in col 64) makes each head's
        ctx matmul deposit sumexp on PSUM row 64. Key chunks in dve_kcs get
        their exp on the vector engine (EXP16) instead of ACT."""
        pcs = [
            psX.tile([DH + 1, NT], F32, tag="pcx", name=f"pcc_{tq}_{hp}_{j}")
            for j in range(2)
        ]
        e2s = [None] * KC
        for kc in range(KC + 1):
            if kc < KC:
                ksl = slice(kc * P, (kc + 1) * P)
                ps2 = psS.tile([P, 2, NT], F32, tag="ps2", name=f"sps_{tq}_{hp}_{kc}")
                e2 = epool.tile([P, 2, NT], BF16, tag="e", name=f"e_{tq}_{hp}_{kc}")
                for j in range(2):  # head 2*hp + j at partition offset 64*j
                    rows = slice(j * DH, (j + 1) * DH)
                    nc.tensor.matmul(
                        ps2[:, j, :], kts[hp][rows, ksl], qt[rows, hp, :],
                        start=True, stop=True,
                    )
                if EXP_SPLIT and kc in dve_kcs:
                    nc.vector._custom_dve(
                        EXP16,
                        out=e2.rearrange("p a n -> p (a n)"),
                        in0=ps2.rearrange("p a n -> p (a n)"),
                        s0=EA0, s1=EA1, imm2=EA2,
                    )
                else:
                    nc.scalar.activation(e2, ps2, AF.Exp, scale=SCALE)
                e2s[kc] = e2
            if filler is not None and kc < KC:
                filler(kc)
            if kc >= 1:
                st, sp = (kc - 1 == 0), (kc - 1 == KC - 1)
                for j in range(2):
                    nc.tensor.matmul(
                        pcs[j],
                        va[:, kc - 1, 2 * hp + j, :],
                        e2s[kc - 1][:, j, :],
                        start=st, stop=sp,
                        skip_group_check=True,
                    )
        for j in range(2):
            # custom-DVE recip can't read PSUM at partition offset 64 on HW;
            # bounce the sumexp row through SBUF first
            se = small.tile([1, NT], F32, tag="sm", name=f"se_{tq}_{hp}_{j}")
            nc.vector.tensor_copy(out=se, in_=pcs[j][DH : DH + 1, :])
            rc = small.tile([1, NT], F32, tag="sm", name=f"rc_{tq}_{hp}_{j}")
            nc.vector.reciprocal_approx_fast(out=rc, in_=se)
            db = chunk.tile([DH, NT], F32, tag="db", name=f"db_{tq}_{hp}_{j}")
            nc.gpsimd.partition_broadcast(db, rc)
            nc.vector.tensor_tensor(
                out=ctx[j * DH : (j + 1) * DH, hp, :],
                in0=pcs[j][0:DH, :],
                in1=db,
                op=OP.mult,
            )

    def oproj_co(tq, ctx, resid, co):
        ts_ = slice(tq * NT, (tq + 1) * NT)
        xqc = chunk.tile([P, NT], F32, tag="xqc", name=f"xqc_{tq}_{co}")
        nc.sync.dma_start(xqc, xq32d[:, co, ts_])
        ps = psC.tile([P, NT], F32, tag="pc", name=f"ops_{tq}_{co}")
        for ki in range(C):
            nc.tensor.matmul(
                ps,
                wo[:, ki, co * P : (co + 1) * P],
                ctx[:, ki, :],
                start=(ki == 0),
                stop=(ki == C - 1),
            )
        nc.vector.scalar_tensor_tensor(
            out=resid[:, co, :],
            in0=ps,
            scalar=bo[:, co : co + 1],
            in1=xqc,
            op0=OP.add,
            op1=OP.add,
        )

    def layernorm(resid, g, be, out_write, tag, out_write_co=None):
        """Normalizes resid IN PLACE (except the final +be which out_write
        directs). resid: [P, C, NT] f32 tile."""
        lnp = psC.tile([P, NT], F32, tag="pc", name=f"lnp_{tag}")
        for co in range(C):
            nc.tensor.matmul(lnp[0:1, :], ones, resid[:, co, :], start=(co == 0),
                             stop=(co == C - 1), skip_group_check=True)
        s4 = stage1.tile([P, C, NT], F32, tag="sq", name=f"sq_{tag}")
        nc.vector.tensor_mul(s4, resid, resid)
        for co in range(C):
            nc.tensor.matmul(lnp[64:65, :], ones, s4[:, co, :], start=(co == 0),
                             stop=(co == C - 1), tile_position=(0, 64),
                             skip_group_check=True)
        mean = small.tile([1, NT], F32, tag="sm", name=f"mean_{tag}")
        nc.vector.tensor_scalar_mul(mean, lnp[0:1, :], 1.0 / D)
        msq = small.tile([1, NT], F32, tag="sm", name=f"msq_{tag}")
        nc.vector.tensor_scalar_mul(msq, lnp[64:65, :], 1.0 / D)
        m2 = small.tile([1, NT], F32, tag="sm", name=f"m2_{tag}")
        nc.vector.tensor_mul(m2, mean, mean)
        var = small.tile([1, NT], F32, tag="sm", name=f"var_{tag}")
        nc.vector.tensor_tensor(out=var, in0=msq, in1=m2, op=OP.subtract)
        # rstd = exp(-0.5 * ln(var + eps)) -- stays in the Exp/Ln ACT table set
        lnv = small.tile([1, NT], F32, tag="sm", name=f"lnv_{tag}")
        nc.scalar.activation(lnv, var, AF.Ln, bias=epst)
        rstd = small.tile([1, NT], F32, tag="sm", name=f"rstd_{tag}")
        nc.scalar.activation(rstd, lnv, AF.Exp, scale=-0.5)
        meanb = chunk.tile([P, NT], F32, tag="bc", name=f"meanb_{tag}")
        nc.gpsimd.partition_broadcast(meanb, mean)
        rstdb = chunk.tile([P, NT], F32, tag="bc", name=f"rstdb_{tag}")
        nc.gpsimd.partition_broadcast(rstdb, rstd)
        if out_write_co is not None:
            for co in range(C):
                nc.vector.tensor_tensor(
                    out=resid[:, co, :], in0=resid[:, co, :], in1=meanb,
                    op=OP.subtract,
                )
                nc.vector.scalar_tensor_tensor(
                    out=resid[:, co, :], in0=resid[:, co, :],
                    scalar=g[:, co : co + 1], in1=rstdb, op0=OP.mult, op1=OP.mult,
                )
                out_write_co(co, resid[:, co, :], be[:, co : co + 1])
            return
        nc.vector.tensor_tensor(
            out=resid, in0=resid,
            in1=meanb[:, None, :].to_broadcast((P, C, NT)), op=OP.subtract,
        )
        nc.vector.tensor_tensor(
            out=resid, in0=resid,
            in1=rstdb[:, None, :].to_broadcast((P, C, NT)), op=OP.mult,
        )
        nc.vector.tensor_tensor(
            out=resid, in0=resid,
            in1=g[:, :, None].to_broadcast((P, C, NT)), op=OP.mult,
        )
        out_write(resid, be)

    # ---------------- P2: attention(tq0); K co=1..3 / V / Q ride inside ---
    ctx0 = ptq.tile([P, C, NT], BF16, tag="ctx", name="ctx_0")

    def make_filler(jobs):
        def f(kc):
            for th in jobs.get(kc, ()):
                th()
        return f

    def J(co, q):
        return lambda: kproj_quarter(co, q)

    def Q(tq, co):
        return lambda: qproj_co(tq, co)

    fill0 = [
        # hp0: V projection paced with the ctx consumer + K/Q chunk-1 prep
        make_filler(
            {
                kc: [
                    *([lambda: load_vr(1)] if kc == 2 else []),
                    lambda kc=kc: vproj_chunk(kc),
                    *([J(1, 0)] if kc == 12 else []),
                    *([Q(0, 1)] if kc == 13 else []),
                ]
                for kc in range(KC)
            }
        ),
        make_filler({0: [J(1, 1)], 4: [J(1, 2)], 8: [J(1, 3)], 11: [J(2, 0)], 13: [Q(0, 2)]}),
        make_filler({0: [J(2, 1)], 4: [J(2, 2)], 8: [J(2, 3)], 11: [J(3, 0)], 13: [Q(0, 3)]}),
        make_filler({0: [J(3, 1)], 4: [J(3, 2)], 8: [J(3, 3)], 12: [Q(1, 0)]}),
    ]
    for hp in range(H // 2):
        if hp == 3:
            qts[1] = ptq.tile([P, C, NT], BF16, tag="qt", name="qt_1")
        attn_hp(0, qts[0], ctx0, hp, filler=fill0[hp])

    # ---------------- P3: attention(tq1) + Oproj(0)/LN1(0) interleaved ----
    ctx1 = ptq.tile([P, C, NT], BF16, tag="ctx", name="ctx_1")
    resid0 = stage.tile([P, C, NT], F32, tag="resid", name="resid_0")
    w1s = []
    ln1f0 = stage.tile([P, C, NT], F32, tag="ln1f", name="ln1f_0")
    ln1b0 = ptq.tile([P, C, NT], BF16, tag="ln1b", name="ln1b_0")

    def write_ln1_0(tt, be):
        nc.vector.tensor_tensor(
            out=ln1f0, in0=tt,
            in1=be[:, :, None].to_broadcast((P, C, NT)), op=OP.add,
        )
        nc.vector.tensor_copy(out=ln1b0, in_=ln1f0)

    for hp in range(H // 2):
        attn_hp(1, qts[1], ctx1, hp)
        # kts[hp] is now dead; its slot hosts the matching quarter of W1
        w1q = ap_.tile([P, FF], BF16, tag=f"big1_{hp}", name=f"w1s_{hp}")
        nc.sync.dma_start(w1q[:, 0 : FF // 2], w1d[:, hp, 0 : FF // 2])
        nc.sync.dma_start(w1q[:, FF // 2 :], w1d[:, hp, FF // 2 :])
        w1s.append(w1q)
        if hp < 3:
            qproj_co(1, hp + 1)  # Q(1) chunk for the next head pair
        if hp < 2:
            # both Oproj(0) chunks per early head pair
            oproj_co(0, ctx0, resid0, 2 * hp)
            oproj_co(0, ctx0, resid0, 2 * hp + 1)
        elif hp == 2:
            # LN1(0): its serial chain hides under the last head pair
            layernorm(resid0, g1, be1, write_ln1_0, "l1_0")

    # ---------------- P4: W2 (reusing the va slot) + tails ----------------
    w2s = ap_.tile([P, CF, D], BF16, tag="big2", name="w2s")
    for kq in range(4):
        nc.sync.dma_start(w2s[:, 4 * kq : 4 * kq + 4, :], w2d[:, 4 * kq : 4 * kq + 4, :])

    # Oproj(1) first; each LN chain then overlaps the next FFN's matmuls
    resid1 = stage.tile([P, C, NT], F32, tag="resid", name="resid_1")
    for co in range(C):
        oproj_co(1, ctx1, resid1, co)

    ln1f1 = stage.tile([P, C, NT], F32, tag="ln1f", name="ln1f_1")
    ln1b1 = ptq.tile([P, C, NT], BF16, tag="ln1b", name="ln1b_1")

    def write_ln1_1(tt, be):
        nc.vector.tensor_tensor(
            out=ln1f1, in0=tt,
            in1=be[:, :, None].to_broadcast((P, C, NT)), op=OP.add,
        )
        nc.vector.tensor_copy(out=ln1b1, in_=ln1f1)

    hbs = [None, None]

    def ffn1(tq, ln1b):
        hb = hpool.tile([P, CF, NT], BF16, tag="h", name=f"h_{tq}")
        for fo in range(CF):
            ps = psC.tile([P, NT], F32, tag="pc", name=f"fps_{tq}_{fo}")
            for ki in range(C):
                nc.tensor.matmul(
                    ps,
                    w1s[ki][:, fo * P : (fo + 1) * P],
                    ln1b[:, ki, :],
                    start=(ki == 0),
                    stop=(ki == C - 1),
                )
            nc.scalar.activation(hb[:, fo, :], ps, AF.Gelu, bias=b1[:, fo : fo + 1])
        hbs[tq] = hb

    def ffn2(tq, ln1f):
        hb = hbs[tq]
        resid2 = stage.tile([P, C, NT], F32, tag="resid", name=f"resid2_{tq}")
        for co in range(C):
            ps = psC.tile([P, NT], F32, tag="pc", name=f"gps_{tq}_{co}")
            for ki in range(CF):
                nc.tensor.matmul(
                    ps,
                    w2s[:, ki, co * P : (co + 1) * P],
                    hb[:, ki, :],
                    start=(ki == 0),
                    stop=(ki == CF - 1),
                )
            nc.vector.scalar_tensor_tensor(
                out=resid2[:, co, :],
                in0=ps,
                scalar=b2[:, co : co + 1],
                in1=ln1f[:, co, :],
                op0=OP.add,
                op1=OP.add,
            )
        return resid2

    def ln2(tq, resid2):
        ts_ = slice(tq * NT, (tq + 1) * NT)

        def write_out_co(co, v, bec, ts_=ts_):
            nc.vector.tensor_scalar(
                out=v, in0=v, scalar1=bec, scalar2=None, op0=OP.add
            )
            nc.sync.dma_start(out_d[:, co, ts_], v)

        layernorm(resid2, g2, be2, None, f"l2_{tq}", out_write_co=write_out_co)

    # LN1(1) first (still on the exp/ln table set), then the whole FFN block
    # on the gelu set (one switch), then LN2s (one switch back). ln2(0) hides
    # under ffn2(1)'s matmuls.
    layernorm(resid1, g1, be1, write_ln1_1, "l1_1")
    ffn1(0, ln1b0)
    r2_0 = ffn2(0, ln1f0)
    ffn1(1, ln1b1)
    ln2(0, r2_0)  # after the last gelu; its ACT chain hides under ffn2(1)
    r2_1 = ffn2(1, ln1f1)
    ln2(1, r2_1)


@functools.lru_cache(maxsize=1)
def build():
    from contextlib import ExitStack

    nc = bacc.Bacc("TRN2", target_bir_lowering=False, debug=False, num_devices=NCORES)
    t = {}

    def din(name, shape, dt):
        t[name] = nc.dram_tensor(name, list(shape), dt, kind="ExternalInput").ap()

    din("xq32", (P, C * TQ), F32)
    din("xqb", (P, C * TQ), BF16)
    din("xkb", (P, C * TK), BF16)
    din("xvb", (P, C * TK), BF16)
    for w in ("wq", "wk", "wv", "wo"):
        din(w, (P, C * D), BF16)
    din("w1", (P, C * FF), BF16)
    din("w2", (P, CF * D), BF16)
    din("ball", (P, 48), F32)
    t["out"] = nc.dram_tensor("out", [D, TQ], F32, kind="ExternalOutput").ap()

    with tile.TileContext(nc) as tc:
        with ExitStack() as es:
            _emit(nc, t, es, tc)
    nc.compile()
    return nc


def make_in_maps(query, key, value, Wq, bq, Wk, bk, Wv, bv, Wo, bo,
                 g1, be1, g2, be2, W1, b1, W2, b2):
    bf = ml_dtypes.bfloat16

    def pmaj(w, dt=bf):
        # [K, N] -> partition-major [128, (K//128) * N], contiguous rows
        w = np.asarray(w)
        k, n = w.shape
        return np.ascontiguousarray(
            w.reshape(k // P, P, n).transpose(1, 0, 2).reshape(P, -1).astype(dt)
        )

    # bv is softmax-invariant through attention: softmax(s)@(V+bv) =
    # softmax(s)@V + bv, so fold it through Wo into bo.
    bo_eff = np.asarray(bv, np.float32) @ np.asarray(Wo, np.float32) + np.asarray(
        bo, np.float32
    )
    cols = [np.asarray(v, np.float32).reshape(-1, P).T for v in (bq, bk, bo_eff, b2, g1, be1, g2, be2, b1)]
    ball = np.ascontiguousarray(np.concatenate(cols, axis=1))  # [128, 48]
    shared = {
        "ball": ball,
        "wq": pmaj(Wq), "wk": pmaj(Wk), "wv": pmaj(Wv), "wo": pmaj(Wo),
        "w1": pmaj(W1), "w2": pmaj(W2),
    }
    in_maps = []
    for core in range(NCORES):
        b, half = divmod(core, 2)
        qsl = slice(half * TQ, (half + 1) * TQ)
        xq_t = np.asarray(query[b, qsl], np.float32).T  # [D, TQ]
        in_maps.append({
            "xq32": pmaj(xq_t, np.float32), "xqb": pmaj(xq_t),
            "xkb": pmaj(np.asarray(key[b], np.float32).T),
            "xvb": pmaj(np.asarray(value[b], np.float32).T), **shared,
        })
    return in_maps


def kernel(**inputs):
    nc = build()
    in_maps = make_in_maps(**inputs)
    res = run_bass_kernel_spmd(nc, in_maps, list(range(NCORES)))
    out = np.empty((B, S, D), np.float32)
    for core in range(NCORES):
        b, half = divmod(core, 2)
        out[b, half * TQ : (half + 1) * TQ] = res.results[core]["out"].T
    return out


if __name__ == "__main__":
    import reference

    inputs = {k: np.asarray(v) for k, v in reference.setup_inputs().items()}
    got = kernel(**inputs)
    exp = np.asarray(reference.reference(**inputs))
    err = np.abs(got - exp).max() / np.abs(exp).max()
    print("rel err:", err)



# revision 30
# speedup vs baseline: 1.2205x; 1.1439x over previous
"""Trainium2 Bass kernel for a cross-modal transformer block (attention + FFN).

Contract: kernel(**inputs) takes the FULL unsharded inputs (numpy, fp32) and
returns the FULL output [4, 2048, 512] fp32.

Sharding: 8 cores = data-parallel over batch (4) x query-sequence halves (2).
Each core computes K/V projections for its batch's full 2048-token sequence
(cheap duplication) so attention needs no collectives.

Device layout: everything feature-major ([features on partitions, tokens on
free]); the host pre-transposes and pre-casts inputs so the device does zero
transposes.
"""

import functools
import sys

import numpy as np

sys.path.insert(0, "/opt/trn_rl_repo")

import ml_dtypes  # noqa: E402

import concourse.bass as bass  # noqa: E402
import concourse.tile as tile  # noqa: E402
from concourse import bacc, mybir  # noqa: E402
from concourse.bass_utils import run_bass_kernel_spmd  # noqa: E402

_orig_tables = bacc.get_activation_tables


def _patched_tables(arch):
    tabs = dict(_orig_tables(arch))
    for name in ("exp_and_others", "exp_and_friends", "natural_log"):
        if name in tabs and "natural_log_exp_and_others" in tabs:
            tabs[name] = set()
    return tabs


bacc.get_activation_tables = _patched_tables

BF16 = mybir.dt.bfloat16
F32 = mybir.dt.float32
AF = mybir.ActivationFunctionType
OP = mybir.AluOpType

B, S, D = 4, 2048, 512
H, DH = 8, 64
FF = 2048
P = 128
C = D // P  # 4 feature chunks
CF = FF // P  # 16 ffn chunks
TQ = S // 2  # 1024 query tokens per core
TK = S  # full key sequence per core
KC = TK // P  # 16 key chunks
NT = 512  # token tile (matmul free dim)
NQ = TQ // NT  # 2 query-token tiles
SCALE = 1.0 / np.sqrt(DH)  # 0.125
LN_EPS = 1e-5
NCORES = 8


def _emit(nc, t, es, tc):
    """Emit the per-core program. t: dict name -> DRAM AP."""
    # ---------------- pools ----------------
    pw = es.enter_context(tc.tile_pool(name="projw", bufs=1))
    wp = es.enter_context(tc.tile_pool(name="w", bufs=1))
    ap_ = es.enter_context(tc.tile_pool(name="acts", bufs=1))
    ptq = es.enter_context(tc.tile_pool(name="ptq", bufs=2))
    psS = es.enter_context(tc.tile_pool(name="psS", bufs=2, space="PSUM"))
    psC = es.enter_context(tc.tile_pool(name="psC", bufs=3, space="PSUM"))
    psE = es.enter_context(tc.tile_pool(name="psE", bufs=1, space="PSUM"))
    stream = es.enter_context(tc.tile_pool(name="stream", bufs=6))
    vpool = es.enter_context(tc.tile_pool(name="vpool", bufs=5))
    stage = es.enter_context(tc.tile_pool(name="stage", bufs=2))
    stage1 = es.enter_context(tc.tile_pool(name="stage1", bufs=1))
    chunk = es.enter_context(tc.tile_pool(name="chunk", bufs=2))
    small = es.enter_context(tc.tile_pool(name="small", bufs=6))
    epool = es.enter_context(tc.tile_pool(name="e", bufs=6))
    hpool = es.enter_context(tc.tile_pool(name="h", bufs=1))

    def ld_w(pool, name, kchunks, n):
        w = pool.tile([P, kchunks, n], BF16, name=name + "_sb")
        src_ = t[name].rearrange("p (c o) -> p c o", c=kchunks)
        for ki in range(kchunks):
            nc.sync.dma_start(w[:, ki, :], src_[:, ki, :])
        return w

    # all small per-feature vectors arrive pre-shuffled in one [P, 48] pack
    ball = wp.tile([P, 48], F32, name="ball")
    nc.sync.dma_start(ball, t["ball"])
    bq, bk, bo, b2 = (ball[:, 4 * i : 4 * (i + 1)] for i in range(4))
    g1, be1, g2, be2 = (ball[:, 16 + 4 * i : 20 + 4 * i] for i in range(4))
    b1 = ball[:, 32:48]

    wk = ld_w(pw, "wk", C, D)
    krs = {}
    for half in range(2):
        kr = []
        for ki in range(C):
            r = stream.tile([P, 1024], BF16, tag="xr", name=f"kr_{half}_{ki}")
            xk_ = t["xkb"].rearrange("p (c q) -> p c q", c=C)
            hs = slice(half * 1024, (half + 1) * 1024)
            nc.sync.dma_start(r[0:64], xk_[0:64, ki, hs])
            nc.sync.dma_start(r[64:P], xk_[64:P, ki, hs])
            kr.append(r)
        krs[half] = kr
    wv = ld_w(pw, "wv", C, D)
    wq = ld_w(pw, "wq", C, D)
    bvb = pw.tile([P, D], F32)
    nc.sync.dma_start(bvb, t["bvb"])

    wo = ld_w(wp, "wo", C, D)
    w1d = t["w1"].rearrange("p (c o) -> p c o", c=C)
    w2d = t["w2"].rearrange("p (c o) -> p c o", c=CF)

    ones = wp.tile([P, 1], F32)
    nc.vector.memset(ones, 1.0)
    onesb = wp.tile([P, 1], BF16)
    nc.vector.memset(onesb, 1.0)
    epst = wp.tile([1, 1], F32)
    nc.vector.memset(epst, LN_EPS)

    # persistent activations (full key sequence); tags shared with the FFN
    # weights, which reuse these slots once attention is done
    kts = [ap_.tile([P, TK], BF16, tag=f"big1_{i}", name=f"kt_{i}") for i in range(C)]
    va = ap_.tile([P, KC, H, DH], BF16, tag="big2", name="va")  # V token-major

    xq32d = t["xq32"].rearrange("p (c q) -> p c q", c=C)
    xqbd = t["xqb"].rearrange("p (c q) -> p c q", c=C)
    xkb = t["xkb"].rearrange("p (c q) -> p c q", c=C)
    xvb = t["xvb"].rearrange("p (c q) -> p c q", c=C)
    out_d = t["out"].rearrange("(c p) q -> p c q", p=P)

    # ---------------- P1: K/V projections + Q proj (tile 0) ----------------
    for half in range(2):
        kr = krs[half]
        for tk in range(2):
            ts_ = slice(half * 1024 + tk * NT, half * 1024 + (tk + 1) * NT)
            tsl = slice(tk * NT, (tk + 1) * NT)
            for co in range(C):
                ps = psC.tile([P, NT], F32, tag="pc", name=f"kps_{half}_{tk}_{co}")
                for ki in range(C):
                    nc.tensor.matmul(
                        ps,
                        wk[:, ki, co * P : (co + 1) * P],
                        kr[ki][:, tsl],
                        start=(ki == 0),
                        stop=(ki == C - 1),
                    )
                nc.vector.tensor_scalar(
                    out=kts[co][:, ts_], in0=ps, scalar1=bk[:, co : co + 1],
                    scalar2=None, op0=OP.add,
                )

    vrs = {}

    def load_vr(half):
        hs = slice(half * 1024, (half + 1) * 1024)
        vr = []
        for ki in range(C):
            r = vpool.tile([P, 1024], BF16, tag="vr", name=f"vr_{half}_{ki}")
            nc.sync.dma_start(r[0:64], xvb[0:64, ki, hs])
            nc.sync.dma_start(r[64:P], xvb[64:P, ki, hs])
            vr.append(r)
        vrs[half] = vr

    def vproj_chunk(tm):
        half, tm8 = divmod(tm, 8)
        msl = slice(tm8 * P, (tm8 + 1) * P)
        vr = vrs[half]
        ps = psC.tile([P, NT], F32, tag="pc", name=f"vps_{tm}")
        for ki in range(C):
            nc.tensor.matmul(
                ps, vr[ki][:, msl], wv[:, ki, :],
                start=(ki == 0), stop=(ki == C - 1),
            )
        nc.vector.tensor_tensor(
            out=va[:, tm, :, :],
            in0=ps.rearrange("p (h d) -> p h d", h=H),
            in1=bvb.rearrange("p (h d) -> p h d", h=H),
            op=OP.add,
        )

    qr = []
    for ki in range(C):
        r = stream.tile([P, TQ], BF16, tag="xr", name=f"qr_{ki}")
        nc.sync.dma_start(r[0:64], xqbd[0:64, ki, :])
        nc.sync.dma_start(r[64:P], xqbd[64:P, ki, :])
        qr.append(r)

    def qproj_co(tq, co):
        tsl = slice(tq * NT, (tq + 1) * NT)
        qt = qts[tq]
        ps = psC.tile([P, NT], F32, tag="pc", name=f"qps_{tq}_{co}")
        for ki in range(C):
            nc.tensor.matmul(
                ps,
                wq[:, ki, co * P : (co + 1) * P],
                qr[ki][:, tsl],
                start=(ki == 0),
                stop=(ki == C - 1),
            )
        nc.vector.tensor_scalar(
            out=qt[:, co, :], in0=ps, scalar1=bq[:, co : co + 1],
            scalar2=None, op0=OP.add,
        )

    def qproj(tq):
        qts[tq] = ptq.tile([P, C, NT], BF16, tag="qt", name=f"qt_{tq}")
        for co in range(C):
            qproj_co(tq, co)
        return qts[tq]

    load_vr(0)
    qts = [None, None]
    qproj(0)

    # ---------------- attention helpers ----------------
    def attn_hp(tq, qt, ctx, hp, filler=None):
        """One head pair's attention, software-pipelined over key chunks.
        filler(kc) lets independent work ride along inside the loop."""
        pcc = psC.tile([P, NT], F32, tag="pc", name=f"pcc_{tq}_{hp}")
        pse = psE.tile([P, NT], F32, tag="pse", name=f"pse_{tq}_{hp}")
        e2s = [None] * KC
        for kc in range(KC + 1):
            if kc < KC:
                ksl = slice(kc * P, (kc + 1) * P)
                ps2 = psS.tile([P, 2, NT], F32, tag="ps2", name=f"sps_{tq}_{hp}_{kc}")
                e2 = epool.tile([P, 2, NT], BF16, tag="e", name=f"e_{tq}_{hp}_{kc}")
                for j in range(2):  # head 2*hp + j at partition offset 64*j
                    rows = slice(j * DH, (j + 1) * DH)
                    nc.tensor.matmul(
                        ps2[:, j, :], kts[hp][rows, ksl], qt[rows, hp, :],
                        start=True, stop=True,
                    )
                nc.scalar.activation(e2, ps2, AF.Exp, scale=SCALE)
                e2s[kc] = e2
            if filler is not None and kc < KC:
                filler(kc)
            if kc >= 1:
                st, sp = (kc - 1 == 0), (kc - 1 == KC - 1)
                for j in range(2):
                    # ctx pair: col-tiled, both heads into one PSUM bank
                    nc.tensor.matmul(
                        pcc[j * DH : (j + 1) * DH, :],
                        va[:, kc - 1, 2 * hp + j, :],
                        e2s[kc - 1][:, j, :],
                        start=st, stop=sp,
                        tile_position=(0, j * DH),
                        skip_group_check=True,
                    )
                for j in range(2):
                    # sumexp accumulators at rows 0 / 64 of a shared bank
                    nc.tensor.matmul(
                        pse[j * DH : j * DH + 1, :],
                        onesb,
                        e2s[kc - 1][:, j, :],
                        start=st, stop=sp,
                        tile_position=(0, j * DH),
                        skip_group_check=True,
                    )
        for j in range(2):
            se = small.tile([1, NT], F32, tag="sm", name=f"se_{tq}_{hp}_{j}")
            nc.vector.tensor_copy(out=se, in_=pse[j * DH : j * DH + 1, :])
            cf = chunk.tile([DH, NT], F32, tag="cf", name=f"cf_{tq}_{hp}_{j}")
            nc.vector.tensor_copy(out=cf, in_=pcc[j * DH : (j + 1) * DH, :])
            rc = small.tile([1, NT], F32, tag="sm", name=f"rc_{tq}_{hp}_{j}")
            nc.vector.reciprocal_approx_fast(out=rc, in_=se)
            db = chunk.tile([DH, NT], F32, tag="db", name=f"db_{tq}_{hp}_{j}")
            nc.gpsimd.partition_broadcast(db, rc)
            nc.vector.tensor_tensor(
                out=ctx[j * DH : (j + 1) * DH, hp, :],
                in0=cf,
                in1=db,
                op=OP.mult,
            )

    def oproj_co(tq, ctx, resid, co):
        ts_ = slice(tq * NT, (tq + 1) * NT)
        xqc = chunk.tile([P, NT], F32, tag="xqc", name=f"xqc_{tq}_{co}")
        nc.sync.dma_start(xqc, xq32d[:, co, ts_])
        ps = psC.tile([P, NT], F32, tag="pc", name=f"ops_{tq}_{co}")
        for ki in range(C):
            nc.tensor.matmul(
                ps,
                wo[:, ki, co * P : (co + 1) * P],
                ctx[:, ki, :],
                start=(ki == 0),
                stop=(ki == C - 1),
            )
        nc.vector.scalar_tensor_tensor(
            out=resid[:, co, :],
            in0=ps,
            scalar=bo[:, co : co + 1],
            in1=xqc,
            op0=OP.add,
            op1=OP.add,
        )

    def layernorm(resid, g, be, out_write, tag, out_write_co=None):
        """Normalizes resid IN PLACE (except the final +be which out_write
        directs). resid: [P, C, NT] f32 tile."""
        lnp = psC.tile([P, NT], F32, tag="pc", name=f"lnp_{tag}")
        for co in range(C):
            nc.tensor.matmul(lnp[0:1, :], ones, resid[:, co, :], start=(co == 0),
                             stop=(co == C - 1), skip_group_check=True)
        s4 = stage1.tile([P, C, NT], F32, tag="sq", name=f"sq_{tag}")
        nc.vector.tensor_mul(s4, resid, resid)
        for co in range(C):
            nc.tensor.matmul(lnp[64:65, :], ones, s4[:, co, :], start=(co == 0),
                             stop=(co == C - 1), tile_position=(0, 64),
                             skip_group_check=True)
        mean = small.tile([1, NT], F32, tag="sm", name=f"mean_{tag}")
        nc.vector.tensor_scalar_mul(mean, lnp[0:1, :], 1.0 / D)
        msq = small.tile([1, NT], F32, tag="sm", name=f"msq_{tag}")
        nc.vector.tensor_scalar_mul(msq, lnp[64:65, :], 1.0 / D)
        m2 = small.tile([1, NT], F32, tag="sm", name=f"m2_{tag}")
        nc.vector.tensor_mul(m2, mean, mean)
        var = small.tile([1, NT], F32, tag="sm", name=f"var_{tag}")
        nc.vector.tensor_tensor(out=var, in0=msq, in1=m2, op=OP.subtract)
        # rstd = exp(-0.5 * ln(var + eps)) -- stays in the Exp/Ln ACT table set
        lnv = small.tile([1, NT], F32, tag="sm", name=f"lnv_{tag}")
        nc.scalar.activation(lnv, var, AF.Ln, bias=epst)
        rstd = small.tile([1, NT], F32, tag="sm", name=f"rstd_{tag}")
        nc.scalar.activation(rstd, lnv, AF.Exp, scale=-0.5)
        meanb = chunk.tile([P, NT], F32, tag="bc", name=f"meanb_{tag}")
        nc.gpsimd.partition_broadcast(meanb, mean)
        rstdb = chunk.tile([P, NT], F32, tag="bc", name=f"rstdb_{tag}")
        nc.gpsimd.partition_broadcast(rstdb, rstd)
        if out_write_co is not None:
            for co in range(C):
                nc.vector.tensor_tensor(
                    out=resid[:, co, :], in0=resid[:, co, :], in1=meanb,
                    op=OP.subtract,
                )
                nc.vector.scalar_tensor_tensor(
                    out=resid[:, co, :], in0=resid[:, co, :],
                    scalar=g[:, co : co + 1], in1=rstdb, op0=OP.mult, op1=OP.mult,
                )
                out_write_co(co, resid[:, co, :], be[:, co : co + 1])
            return
        nc.vector.tensor_tensor(
            out=resid, in0=resid,
            in1=meanb[:, None, :].to_broadcast((P, C, NT)), op=OP.subtract,
        )
        nc.vector.tensor_tensor(
            out=resid, in0=resid,
            in1=rstdb[:, None, :].to_broadcast((P, C, NT)), op=OP.mult,
        )
        nc.vector.tensor_tensor(
            out=resid, in0=resid,
            in1=g[:, :, None].to_broadcast((P, C, NT)), op=OP.mult,
        )
        out_write(resid, be)

    # ---------------- P2: attention(tq0) with V-proj / Qproj(tq1) riding --
    ctx0 = ptq.tile([P, C, NT], BF16, tag="ctx", name="ctx_0")

    def fill_hp0(kc):
        # V chunk kc feeds this very head pair's ctx matmuls a beat later
        if kc == 2:
            load_vr(1)
        vproj_chunk(kc)

    def fill_hp2(kc):
        if kc % 4 == 0:
            qproj_co(1, kc // 4)

    for hp in range(H // 2):
        attn_hp(0, qts[0], ctx0, hp, filler=(fill_hp0 if hp == 0 else fill_hp2 if hp == 2 else None))
        if hp == 1:
            qts[1] = ptq.tile([P, C, NT], BF16, tag="qt", name="qt_1")

    # ---------------- P3: attention(tq1) + Oproj(0)/LN1(0) interleaved ----
    ctx1 = ptq.tile([P, C, NT], BF16, tag="ctx", name="ctx_1")
    resid0 = stage.tile([P, C, NT], F32, tag="resid", name="resid_0")
    w1s = []
    ln1f0 = stage.tile([P, C, NT], F32, tag="ln1f", name="ln1f_0")
    ln1b0 = ptq.tile([P, C, NT], BF16, tag="ln1b", name="ln1b_0")

    def write_ln1_0(tt, be):
        nc.vector.tensor_tensor(
            out=ln1f0, in0=tt,
            in1=be[:, :, None].to_broadcast((P, C, NT)), op=OP.add,
        )
        nc.vector.tensor_copy(out=ln1b0, in_=ln1f0)

    for hp in range(H // 2):
        attn_hp(1, qts[1], ctx1, hp)
        # kts[hp] is now dead; its slot hosts the matching quarter of W1
        w1q = ap_.tile([P, FF], BF16, tag=f"big1_{hp}", name=f"w1s_{hp}")
        nc.sync.dma_start(w1q[:, 0 : FF // 2], w1d[:, hp, 0 : FF // 2])
        nc.sync.dma_start(w1q[:, FF // 2 :], w1d[:, hp, FF // 2 :])
        w1s.append(w1q)
        if hp < 2:
            # both Oproj(0) chunks per early head pair
            oproj_co(0, ctx0, resid0, 2 * hp)
            oproj_co(0, ctx0, resid0, 2 * hp + 1)
        elif hp == 2:
            # LN1(0): its serial chain hides under the last head pair
            layernorm(resid0, g1, be1, write_ln1_0, "l1_0")

    # ---------------- P4: W2 (reusing the va slot) + tails ----------------
    w2s = ap_.tile([P, CF, D], BF16, tag="big2", name="w2s")
    for kq in range(4):
        nc.sync.dma_start(w2s[:, 4 * kq : 4 * kq + 4, :], w2d[:, 4 * kq : 4 * kq + 4, :])

    # Oproj(1) first; each LN chain then overlaps the next FFN's matmuls
    resid1 = stage.tile([P, C, NT], F32, tag="resid", name="resid_1")
    for co in range(C):
        oproj_co(1, ctx1, resid1, co)

    ln1f1 = stage.tile([P, C, NT], F32, tag="ln1f", name="ln1f_1")
    ln1b1 = ptq.tile([P, C, NT], BF16, tag="ln1b", name="ln1b_1")

    def write_ln1_1(tt, be):
        nc.vector.tensor_tensor(
            out=ln1f1, in0=tt,
            in1=be[:, :, None].to_broadcast((P, C, NT)), op=OP.add,
        )
        nc.vector.tensor_copy(out=ln1b1, in_=ln1f1)

    hbs = [None, None]

    def ffn1(tq, ln1b):
        hb = hpool.tile([P, CF, NT], BF16, tag="h", name=f"h_{tq}")
        for fo in range(CF):
            ps = psC.tile([P, NT], F32, tag="pc", name=f"fps_{tq}_{fo}")
            for ki in range(C):
                nc.tensor.matmul(
                    ps,
                    w1s[ki][:, fo * P : (fo + 1) * P],
                    ln1b[:, ki, :],
                    start=(ki == 0),
                    stop=(ki == C - 1),
                )
            nc.scalar.activation(hb[:, fo, :], ps, AF.Gelu, bias=b1[:, fo : fo + 1])
        hbs[tq] = hb

    def ffn2(tq, ln1f):
        hb = hbs[tq]
        resid2 = stage1.tile([P, C, NT], F32, tag="resid2", name=f"resid2_{tq}")
        for co in range(C):
            ps = psC.tile([P, NT], F32, tag="pc", name=f"gps_{tq}_{co}")
            for ki in range(CF):
                nc.tensor.matmul(
                    ps,
                    w2s[:, ki, co * P : (co + 1) * P],
                    hb[:, ki, :],
                    start=(ki == 0),
                    stop=(ki == CF - 1),
                )
            nc.vector.scalar_tensor_tensor(
                out=resid2[:, co, :],
                in0=ps,
                scalar=b2[:, co : co + 1],
                in1=ln1f[:, co, :],
                op0=OP.add,
                op1=OP.add,
            )
        return resid2

    def ln2(tq, resid2):
        ts_ = slice(tq * NT, (tq + 1) * NT)

        def write_out_co(co, v, bec, ts_=ts_):
            nc.vector.tensor_scalar(
                out=v, in0=v, scalar1=bec, scalar2=None, op0=OP.add
            )
            nc.sync.dma_start(out_d[:, co, ts_], v)

        layernorm(resid2, g2, be2, None, f"l2_{tq}", out_write_co=write_out_co)

    ffn1(0, ln1b0)                     # fills the Oproj(1) tail
    layernorm(resid1, g1, be1, write_ln1_1, "l1_1")  # chain hides in FFN2(0)
    r2_0 = ffn2(0, ln1f0)
    ln2(0, r2_0)                       # chain hides in FFN1(1)
    ffn1(1, ln1b1)
    r2_1 = ffn2(1, ln1f1)
    ln2(1, r2_1)


@functools.lru_cache(maxsize=1)
def build():
    from contextlib import ExitStack

    nc = bacc.Bacc("TRN2", target_bir_lowering=False, debug=False, num_devices=NCORES)
    t = {}

    def din(name, shape, dt):
        t[name] = nc.dram_tensor(name, list(shape), dt, kind="ExternalInput").ap()

    din("xq32", (P, C * TQ), F32)
    din("xqb", (P, C * TQ), BF16)
    din("xkb", (P, C * TK), BF16)
    din("xvb", (P, C * TK), BF16)
    for w in ("wq", "wk", "wv", "wo"):
        din(w, (P, C * D), BF16)
    din("w1", (P, C * FF), BF16)
    din("w2", (P, CF * D), BF16)
    din("ball", (P, 48), F32)
    din("bvb", (P, D), F32)
    t["out"] = nc.dram_tensor("out", [D, TQ], F32, kind="ExternalOutput").ap()

    with tile.TileContext(nc) as tc:
        with ExitStack() as es:
            _emit(nc, t, es, tc)
    nc.compile()
    return nc


def make_in_maps(query, key, value, Wq, bq, Wk, bk, Wv, bv, Wo, bo,
                 g1, be1, g2, be2, W1, b1, W2, b2):
    bf = ml_dtypes.bfloat16

    def pmaj(w, dt=bf):
        # [K, N] -> partition-major [128, (K//128) * N], contiguous rows
        w = np.asarray(w)
        k, n = w.shape
        return np.ascontiguousarray(
            w.reshape(k // P, P, n).transpose(1, 0, 2).reshape(P, -1).astype(dt)
        )

    cols = [np.asarray(v, np.float32).reshape(-1, P).T for v in (bq, bk, bo, b2, g1, be1, g2, be2, b1)]
    ball = np.ascontiguousarray(np.concatenate(cols, axis=1))  # [128, 48]
    bvb = np.ascontiguousarray(np.broadcast_to(np.asarray(bv, np.float32), (P, D)))
    shared = {
        "ball": ball, "bvb": bvb,
        "wq": pmaj(Wq), "wk": pmaj(Wk), "wv": pmaj(Wv), "wo": pmaj(Wo),
        "w1": pmaj(W1), "w2": pmaj(W2),
    }
    in_maps = []
    for core in range(NCORES):
        b, half = divmod(core, 2)
        qsl = slice(half * TQ, (half + 1) * TQ)
        xq_t = np.asarray(query[b, qsl], np.float32).T  # [D, TQ]
        in_maps.append({
            "xq32": pmaj(xq_t, np.float32), "xqb": pmaj(xq_t),
            "xkb": pmaj(np.asarray(key[b], np.float32).T),
            "xvb": pmaj(np.asarray(value[b], np.float32).T), **shared,
        })
    return in_maps


def kernel(**inputs):
    nc = build()
    in_maps = make_in_maps(**inputs)
    res = run_bass_kernel_spmd(nc, in_maps, list(range(NCORES)))
    out = np.empty((B, S, D), np.float32)
    for core in range(NCORES):
        b, half = divmod(core, 2)
        out[b, half * TQ : (half + 1) * TQ] = res.results[core]["out"].T
    return out


if __name__ == "__main__":
    import reference

    inputs = {k: np.asarray(v) for k, v in reference.setup_inputs().items()}
    got = kernel(**inputs)
    exp = np.asarray(reference.reference(**inputs))
    err = np.abs(got - exp).max() / np.abs(exp).max()
    print("rel err:", err)

